# revision 27
# baseline (speedup 1.0000x reference)
"""Self-contained Trainium2 Bass kernel for the sparse point-attention module.

Strategy: shard the point dimension n across the 8 NeuronCores (512 points
each, both batch entries on every core).  Each core gets the full `pos`
(tiny) so the KNN is purely local; everything else is data-parallel and no
collectives are needed.

Per-core pipeline (per batch b):
  Phase A (all 4096 points):  qfull = Wq@pos+bq,  pefull = MLP_p(qfull)
  Phase B (per 128-query tile):
     - dneg[i,j] = 2 p_i.p_j - |p_j|^2 via one fp32r matmul (K=4 with a
       ones row); top-16 neighbours via DVE max8 / max_index / match_replace
       (tie behaviour matches jax.lax.top_k exactly).
     - neighbour q/pe columns fetched from qfull/pefull with gpsimd
       ap_gather (the 16-NN index list is rewrapped with a PE transpose +
       tiled-identity broadcast matmul).
     - k_f/v convs, a-branch MLP (bf16 matmuls), softmax over the 16
       neighbours expressed as (sum e*w)/(sum e), final 1x1 conv.

Algebraic folds done on the host: BN (eval mode) into Wa1/Wp1+biases;
qk_rel*pe+pe = (q - k_f + 1)*pe with (1 - bk) folded into the k_f eviction;
ba2 dropped (softmax-invariant); all weights pre-transposed for the PE.
"""

import numpy as np
import ml_dtypes

BF16 = ml_dtypes.bfloat16

# ---- problem dimensions (hardcoded, must match the grader's inputs) ----
B = 2
CIN = 128
N = 4096
KK = 16          # neighbours
DIM = 256
PHID = 64
AHID = 1024
NCORES = 8
NLOC = N // NCORES
BN_EPS = 1e-5
NEG_BIG = -1e30


def _dims_full():
    return dict(B=B, CIN=CIN, N=N, KK=KK, DIM=DIM, PHID=PHID, AHID=AHID,
                NLOC=NLOC)


def build_nc(dims):
    """Build the (single, SPMD) Bass program for one core's shard."""
    import concourse.bass as bass
    import concourse.mybir as mybir
    import concourse.tile as tile
    from concourse import bacc
    from concourse.bass import ts

    fp32 = mybir.dt.float32
    fp32r = mybir.dt.float32r
    bf16 = mybir.dt.bfloat16
    u16 = mybir.dt.uint16
    i16 = mybir.dt.int16
    AF = mybir.ActivationFunctionType
    OP = mybir.AluOpType
    AX = mybir.AxisListType

    Bn = dims["B"]; CINn = dims["CIN"]; Nn = dims["N"]; KKn = dims["KK"]
    DIMn = dims["DIM"]; PHIDn = dims["PHID"]; AHIDn = dims["AHID"]
    NLOCn = dims["NLOC"]

    QT = min(128, NLOCn)              # queries per KNN tile
    NQT = NLOCn // QT                 # KNN tiles per batch
    CHUNK = 512                       # matmul column chunk (n,k cols)
    CQ = CHUNK // KKn                 # queries per chunk (32)
    NCH_TILE = (QT * KKn) // CHUNK    # chunks per KNN tile
    NCH_D = Nn // 512                 # 512-col chunks of the distance row
    DM = DIMn // 128                  # feature tiles (2)
    AM = AHIDn // 128                 # a-hidden tiles (8)
    KA1 = DIMn // 128                 # contraction tiles for a1 (2)

    nc = bacc.Bacc()

    # ---- DRAM parameters ----
    key_r = nc.declare_dram_parameter("key_r", [Bn, CINn, NLOCn * KKn], bf16, isOutput=False)
    val_r = nc.declare_dram_parameter("val_r", [Bn, CINn, NLOCn * KKn], bf16, isOutput=False)
    paug_lhs = nc.declare_dram_parameter("paug_lhs", [Bn, 11, NLOCn], bf16, isOutput=False)
    paug_rhs = nc.declare_dram_parameter("paug_rhs", [Bn, 11, Nn], bf16, isOutput=False)
    pq_rhs = nc.declare_dram_parameter("pq_rhs", [Bn, 4, Nn], bf16, isOutput=False)
    WkTn_d = nc.declare_dram_parameter("WkTn", [CINn, DIMn], bf16, isOutput=False)
    WvT_d = nc.declare_dram_parameter("WvT", [CINn, DIMn], bf16, isOutput=False)
    WqTb_d = nc.declare_dram_parameter("WqTb", [4, DIMn], bf16, isOutput=False)
    Wp1T_d = nc.declare_dram_parameter("Wp1T", [128, DM, PHIDn], bf16, isOutput=False)
    Wp2T_d = nc.declare_dram_parameter("Wp2T", [PHIDn, DIMn], bf16, isOutput=False)
    Wa1T_d = nc.declare_dram_parameter("Wa1T", [128, KA1, AHIDn], bf16, isOutput=False)
    Wa2T_d = nc.declare_dram_parameter("Wa2T", [128, AM, DIMn], bf16, isOutput=False)
    WeT_d = nc.declare_dram_parameter("WeT", [128, DM, DIMn], bf16, isOutput=False)
    negbk1_d = nc.declare_dram_parameter("negbk1", [128, DM], fp32, isOutput=False)
    bv_d = nc.declare_dram_parameter("bvf", [128, DM], fp32, isOutput=False)
    bp1_d = nc.declare_dram_parameter("bp1f", [PHIDn, 1], fp32, isOutput=False)
    bp2_d = nc.declare_dram_parameter("bp2f", [128, DM], fp32, isOutput=False)
    ba1_d = nc.declare_dram_parameter("ba1f", [128, AM], fp32, isOutput=False)
    be_d = nc.declare_dram_parameter("bef", [128, DM], fp32, isOutput=False)
    teye_d = nc.declare_dram_parameter("teye16", [16, 128], fp32, isOutput=False)
    eye_d = nc.declare_dram_parameter("eye128", [128, 128], fp32, isOutput=False)
    out_d = nc.declare_dram_parameter("out", [Bn, DIMn, NLOCn], fp32, isOutput=True)

    with tile.TileContext(nc) as tc:
        with (
            tc.tile_pool(name="wpool", bufs=1) as wpool,
            tc.tile_pool(name="bpool", bufs=1) as bpool,
            tc.tile_pool(name="dpool", bufs=1) as dpool,
            tc.tile_pool(name="kpool", bufs=2) as kpool,
            tc.tile_pool(name="cpool", bufs=2) as cpool,
            tc.tile_pool(name="c1pool", bufs=1) as c1pool,
            tc.tile_pool(name="papool", bufs=2) as papool,
            tc.tile_pool(name="ps", bufs=8, space="PSUM") as ps,
        ):
            # ---- load weights / constants once ----
            WkTn = wpool.tile([CINn, DIMn], bf16)
            WvT = wpool.tile([CINn, DIMn], bf16)
            WqTb = wpool.tile([4, DIMn], bf16)
            Wp1T = wpool.tile([128, DM, PHIDn], bf16)
            Wp2T = wpool.tile([PHIDn, DIMn], bf16)
            Wa1T = wpool.tile([128, KA1, AHIDn], bf16)
            Wa2T = wpool.tile([128, AM, DIMn], bf16)
            WeT = wpool.tile([128, DM, DIMn], bf16)
            negbk1 = wpool.tile([128, DM], fp32)
            bvf = wpool.tile([128, DM], fp32)
            bp1f = wpool.tile([PHIDn, 1], fp32)
            bp2f = wpool.tile([128, DM], fp32)
            ba1f = wpool.tile([128, AM], fp32)
            bef = wpool.tile([128, DM], fp32)
            teye16 = wpool.tile([16, 128], fp32)
            eye128 = wpool.tile([128, 128], fp32)
            for sb, dr in [(WkTn, WkTn_d), (WvT, WvT_d), (WqTb, WqTb_d),
                           (Wp1T, Wp1T_d), (Wp2T, Wp2T_d), (Wa1T, Wa1T_d),
                           (Wa2T, Wa2T_d), (WeT, WeT_d), (negbk1, negbk1_d),
                           (bvf, bv_d), (bp1f, bp1_d), (bp2f, bp2_d),
                           (ba1f, ba1_d), (bef, be_d), (teye16, teye_d),
                           (eye128, eye_d)]:
                nc.sync.dma_start(out=sb[:], in_=dr[:])

            for b in range(Bn):
                # ============== Phase A: qfull / pefull on all N points =====
                pq_sb = papool.tile([4, Nn], bf16, tag="pq_sb")
                nc.sync.dma_start(out=pq_sb[:], in_=pq_rhs[b])
                prhs_sb = papool.tile([11, Nn], bf16, tag="prhs_sb")
                nc.sync.dma_start(out=prhs_sb[:], in_=paug_rhs[b])
                plhs_sb = papool.tile([11, NLOCn], bf16, tag="plhs_sb")
                nc.sync.dma_start(out=plhs_sb[:], in_=paug_lhs[b])

                qf32 = bpool.tile([128, DM, Nn], fp32, tag="qf32")
                qbf = bpool.tile([128, DM, Nn], bf16, tag="qbf")
                pef32 = bpool.tile([128, DM, Nn], fp32, tag="pef32")
                pe1sb = bpool.tile([PHIDn, Nn], bf16, tag="pe1sb")
                aggsb = bpool.tile([128, DM, NLOCn], bf16, tag="aggsb")

                for nch in range(Nn // 512):
                    for m in range(DM):
                        qps = ps.tile([128, 512], fp32, tag="ps")
                        nc.tensor.matmul(qps[:], WqTb[:, ts(m, 128)],
                                         pq_sb[:, ts(nch, 512)])
                        nc.scalar.activation(qf32[:, m, ts(nch, 512)], qps[:],
                                             AF.Copy)
                        nc.vector.tensor_copy(qbf[:, m, ts(nch, 512)], qps[:])
                for nch in range(Nn // 512):
                    p1ps = ps.tile([PHIDn, 512], fp32, tag="ps")
                    for kt in range(DM):
                        nc.tensor.matmul(
                            p1ps[:], Wp1T[:, kt, :],
                            qbf[:, kt, ts(nch, 512)],
                            start=(kt == 0), stop=(kt == DM - 1))
                    nc.scalar.activation(pe1sb[:, ts(nch, 512)], p1ps[:],
                                         AF.Relu, bias=bp1f[:, 0:1])
                for nch in range(Nn // 512):
                    for m in range(DM):
                        p2ps = ps.tile([128, 512], fp32, tag="ps")
                        nc.tensor.matmul(p2ps[:], Wp2T[:, ts(m, 128)],
                                         pe1sb[:, ts(nch, 512)])
                        nc.vector.tensor_scalar_add(pef32[:, m, ts(nch, 512)],
                                                    p2ps[:], bp2f[:, m:m + 1])

                # ============== Phase B: per query tile ======================
                for t in range(NQT):
                    # ---- distance rows (negated, +sq_j folded in) ----
                    dsb = dpool.tile([QT, Nn], fp32, tag="dsb")
                    for nch in range(NCH_D):
                        dps = ps.tile([QT, 512], fp32, tag="ps")
                        nc.tensor.matmul(
                            dps[:], plhs_sb[:, ts(t, QT)],
                            prhs_sb[:, ts(nch, 512)])
                        nc.scalar.activation(dsb[:, ts(nch, 512)], dps[:],
                                             AF.Copy)

                    # ---- top-16 neighbours ----
                    v8a = kpool.tile([QT, 8], fp32, tag="v8a")
                    v8b = kpool.tile([QT, 8], fp32, tag="v8b")
                    idxg = kpool.tile([QT, 16], u16, tag="idxg")
                    idxf = kpool.tile([QT, 16], fp32, tag="idxf")
                    idxT = kpool.tile([16, QT], fp32, tag="idxT")
                    idxw = kpool.tile([128, QT], i16, tag="idxw")
                    if dims.get("debug") == "noknn":
                        nc.vector.memset(idxf[:], 1.0)
                    else:
                        nc.vector.max(out=v8a[:], in_=dsb[:])
                        nc.vector.max_index(out=idxg[:, 0:8], in_max=v8a[:],
                                            in_values=dsb[:])
                        nc.vector.match_replace(out=dsb[:], in_to_replace=v8a[:],
                                                in_values=dsb[:],
                                                imm_value=NEG_BIG)
                        nc.vector.max(out=v8b[:], in_=dsb[:])
                        nc.vector.max_index(out=idxg[:, 8:16], in_max=v8b[:],
                                            in_values=dsb[:])
                        nc.vector.tensor_copy(idxf[:], idxg[:])
                    trps = ps.tile([16, QT], fp32, tag="ps")
                    nc.tensor.transpose(trps[:], idxf[:], eye128[:QT, :QT])
                    nc.vector.tensor_copy(idxT[:], trps[:])
                    bcps = ps.tile([128, QT], fp32, tag="ps")
                    nc.tensor.matmul(bcps[:], teye16[:], idxT[:])
                    nc.vector.tensor_copy(idxw[:], bcps[:])

                    for c in range(NCH_TILE):
                        gc = t * NCH_TILE + c          # global chunk index
                        col0 = gc * CHUNK
                        # ---- gather neighbour q / pe columns ----
                        qg = cpool.tile([128, DM, CHUNK], fp32, tag="qg")
                        peg = cpool.tile([128, DM, CHUNK], fp32, tag="peg")
                        if dims.get("debug") == "nogather":
                            for m in range(DM):
                                nc.vector.tensor_copy(
                                    qg[:, m, :], qf32[:, m, 0:CHUNK])
                                nc.vector.tensor_copy(
                                    peg[:, m, :], pef32[:, m, 0:CHUNK])
                        else:
                            for m in range(DM):
                                nc.gpsimd.ap_gather(
                                    qg[:, m, :], qf32[:, m, :],
                                    idxw[:, c * CQ:(c + 1) * CQ],
                                    channels=128, num_elems=Nn, d=1,
                                    num_idxs=CHUNK)
                                nc.gpsimd.ap_gather(
                                    peg[:, m, :], pef32[:, m, :],
                                    idxw[:, c * CQ:(c + 1) * CQ],
                                    channels=128, num_elems=Nn, d=1,
                                    num_idxs=CHUNK)

                        # ---- k_f / v convs ----
                        kbf = cpool.tile([CINn, CHUNK], bf16, tag="kbf")
                        vbf = cpool.tile([CINn, CHUNK], bf16, tag="vbf")
                        nc.sync.dma_start(out=kbf[:],
                                          in_=key_r[b, :, col0:col0 + CHUNK])
                        nc.sync.dma_start(out=vbf[:],
                                          in_=val_r[b, :, col0:col0 + CHUNK])

                        rr = c1pool.tile([128, DM, CHUNK], fp32, tag="rr")
                        vpe = c1pool.tile([128, DM, CHUNK], fp32, tag="vpe")
                        a1in = c1pool.tile([128, DM, CHUNK], bf16, tag="a1in")
                        for m in range(DM):
                            kfps = ps.tile([128, CHUNK], fp32, tag="ps")
                            nc.tensor.matmul(kfps[:], WkTn[:, ts(m, 128)], kbf[:])
                            # r = (-(Wk@key) + (1-bk)) + q_g
                            nc.vector.scalar_tensor_tensor(
                                rr[:, m, :], kfps[:], negbk1[:, m:m + 1],
                                qg[:, m, :], op0=OP.add, op1=OP.add)
                            vps = ps.tile([128, CHUNK], fp32, tag="ps")
                            nc.tensor.matmul(vps[:], WvT[:, ts(m, 128)], vbf[:])
                            nc.vector.scalar_tensor_tensor(
                                vpe[:, m, :], vps[:], bvf[:, m:m + 1],
                                peg[:, m, :], op0=OP.add, op1=OP.add)
                            nc.vector.tensor_mul(a1in[:, m, :], rr[:, m, :],
                                                 peg[:, m, :])

                        # ---- a-branch MLP ----
                        a1r = c1pool.tile([128, AM, CHUNK], bf16, tag="a1r")
                        for mt in range(AM):
                            a1ps = ps.tile([128, CHUNK], fp32, tag="ps")
                            for kt in range(KA1):
                                nc.tensor.matmul(
                                    a1ps[:], Wa1T[:, kt, ts(mt, 128)],
                                    a1in[:, kt, :],
                                    start=(kt == 0), stop=(kt == KA1 - 1))
                            nc.scalar.activation(a1r[:, mt, :], a1ps[:],
                                                 AF.Relu, bias=ba1f[:, mt:mt + 1])

                        ee = cpool.tile([128, DM, CHUNK], fp32, tag="ee")
                        esum = cpool.tile([128, DM, CQ], fp32, tag="esum")
                        erec = cpool.tile([128, DM, CQ], fp32, tag="erec")
                        aggc = cpool.tile([128, DM, CQ], fp32, tag="aggc")
                        for m in range(DM):
                            a2ps = ps.tile([128, CHUNK], fp32, tag="ps")
                            for kt in range(AM):
                                nc.tensor.matmul(
                                    a2ps[:], Wa2T[:, kt, ts(m, 128)],
                                    a1r[:, kt, :],
                                    start=(kt == 0), stop=(kt == AM - 1))
                            # softmax over the 16 neighbours as sum(e*w)/sum(e)
                            nc.scalar.activation(ee[:, m, :], a2ps[:], AF.Exp)
                            nc.vector.tensor_reduce(
                                esum[:, m, :],
                                ee[:, m, :].rearrange("p (g k) -> p g k", k=KKn),
                                axis=AX.X, op=OP.add)
                            nc.vector.reciprocal(erec[:, m, :], esum[:, m, :])
                            nc.vector.tensor_tensor(ee[:, m, :], ee[:, m, :],
                                                    vpe[:, m, :], op=OP.mult)
                            nc.vector.tensor_reduce(
                                aggc[:, m, :],
                                ee[:, m, :].rearrange("p (g k) -> p g k", k=KKn),
                                axis=AX.X, op=OP.add)
                            nc.vector.tensor_mul(
                                aggsb[:, m, t * QT + c * CQ:t * QT + (c + 1) * CQ],
                                aggc[:, m, :], erec[:, m, :])

                # ---- final 1x1 conv over aggregated features ----
                for nloc0 in range(0, NLOCn, 512):
                    w = min(512, NLOCn - nloc0)
                    for m in range(DM):
                        yps = ps.tile([128, 512], fp32, tag="ps")
                        for kt in range(DM):
                            nc.tensor.matmul(
                                yps[:, :w], WeT[:, kt, ts(m, 128)],
                                aggsb[:, kt, nloc0:nloc0 + w],
                                start=(kt == 0), stop=(kt == DM - 1))
                        yev = cpool.tile([128, 512], fp32, tag="yev")
                        nc.vector.tensor_scalar_add(yev[:, :w], yps[:, :w],
                                                    bef[:, m:m + 1])
                        nc.sync.dma_start(
                            out=out_d[b, ts(m, 128), nloc0:nloc0 + w],
                            in_=yev[:, :w])

    nc.finalize()   # Bacc.finalize: wait legalization, library loads, ISA codegen
    return nc


def host_prepare(inputs, dims, ncores=NCORES):
    """Fold BN/biases into weights, pre-transpose for the PE, shard by n."""
    d = dims
    f32 = np.float32
    key = np.asarray(inputs["key"], f32)
    values = np.asarray(inputs["values"], f32)
    pos = np.asarray(inputs["pos"], f32)
    g = lambda n: np.asarray(inputs[n], f32)

    Wk, bk = g("Wk"), g("bk")
    Wq, bq = g("Wq"), g("bq")
    Wv, bv = g("Wv"), g("bv")
    Wp1, bp1 = g("Wp1"), g("bp1")
    Wp2, bp2 = g("Wp2"), g("bp2")
    Wa1, ba1 = g("Wa1"), g("ba1")
    Wa2 = g("Wa2")
    We, be = g("We"), g("be")

    p_sc = g("p_gamma") / np.sqrt(g("p_var") + f32(BN_EPS))
    Wp1f = (Wp1 * p_sc[:, None]).astype(f32)
    bp1f = (bp1 * p_sc + g("p_beta") - g("p_mean") * p_sc).astype(f32)
    a_sc = g("a_gamma") / np.sqrt(g("a_var") + f32(BN_EPS))
    Wa1f = (Wa1 * a_sc[:, None]).astype(f32)
    ba1f = (ba1 * a_sc + g("a_beta") - g("a_mean") * a_sc).astype(f32)

    DM = d["DIM"] // 128
    AM = d["AHID"] // 128
    KA1 = d["DIM"] // 128

    def colsplit(v, nt):  # (nt*128,) -> (128, nt)
        return np.ascontiguousarray(v.reshape(nt, 128).T).astype(f32)

    common = {
        "WkTn": np.ascontiguousarray((-Wk).T).astype(BF16),
        "WvT": np.ascontiguousarray(Wv.T).astype(BF16),
        "WqTb": np.ascontiguousarray(
            np.concatenate([Wq.T, bq[None, :]], 0)).astype(BF16),
        "Wp1T": np.ascontiguousarray(
            Wp1f.T.reshape(KA1, 128, d["PHID"]).transpose(1, 0, 2)).astype(BF16),
        "Wp2T": np.ascontiguousarray(Wp2.T).astype(BF16),
        "Wa1T": np.ascontiguousarray(
            Wa1f.T.reshape(KA1, 128, d["AHID"]).transpose(1, 0, 2)).astype(BF16),
        "Wa2T": np.ascontiguousarray(
            Wa2.T.reshape(AM, 128, d["DIM"]).transpose(1, 0, 2)).astype(BF16),
        "WeT": np.ascontiguousarray(
            We.T.reshape(DM, 128, d["DIM"]).transpose(1, 0, 2)).astype(BF16),
        "negbk1": colsplit((1.0 - bk).astype(f32), DM),
        "bvf": colsplit(bv, DM),
        "bp1f": bp1f.reshape(d["PHID"], 1).astype(f32),
        "bp2f": colsplit(bp2, DM),
        "ba1f": colsplit(ba1f, AM),
        "bef": colsplit(be, DM),
        "teye16": np.ascontiguousarray(
            np.tile(np.eye(16, dtype=f32), (1, 8))),
        "eye128": np.eye(128, dtype=f32),
    }

    # hi/lo bf16 split of pos and |p|^2 for the exact-enough distance matmul:
    # dneg = 2(hi_i+lo_i).(hi_j+lo_j) - sq_j, dropping only the lo.lo term.
    sq = (pos * pos).sum(axis=1).astype(f32)              # (B, N)
    pos_hi = pos.astype(BF16)
    pos_lo = (pos - pos_hi.astype(f32)).astype(BF16)
    sq_hi = sq.astype(BF16)
    sq_lo = (sq - sq_hi.astype(f32)).astype(BF16)
    ones_n = np.ones((d["B"], 1, d["N"]), f32)
    paug_rhs = np.concatenate(
        [2.0 * pos_hi.astype(f32), 2.0 * pos_lo.astype(f32),
         2.0 * pos_hi.astype(f32), -sq_hi.astype(f32)[:, None, :],
         -sq_lo.astype(f32)[:, None, :]], 1).astype(BF16)
    pq_rhs = np.concatenate([pos, ones_n], 1).astype(BF16)

    in_maps = []
    for cid in range(ncores):
        n0 = cid * d["NLOC"]
        n1 = n0 + d["NLOC"]
        m = dict(common)
        m["key_r"] = np.ascontiguousarray(key[:, :, n0:n1, :]).reshape(
            d["B"], d["CIN"], d["NLOC"] * d["KK"]).astype(BF16)
        m["val_r"] = np.ascontiguousarray(values[:, :, n0:n1, :]).reshape(
            d["B"], d["CIN"], d["NLOC"] * d["KK"]).astype(BF16)
        m["paug_lhs"] = np.ascontiguousarray(np.concatenate(
            [pos_hi.astype(f32)[:, :, n0:n1], pos_hi.astype(f32)[:, :, n0:n1],
             pos_lo.astype(f32)[:, :, n0:n1],
             np.ones((d["B"], 2, d["NLOC"]), f32)], 1)).astype(BF16)
        m["paug_rhs"] = paug_rhs
        m["pq_rhs"] = pq_rhs
        in_maps.append(m)
    return in_maps


_NC_CACHE = {}


def _get_nc(dims_key):
    if dims_key not in _NC_CACHE:
        _NC_CACHE[dims_key] = build_nc(_dims_full())
    return _NC_CACHE[dims_key]


def kernel(**inputs):
    from concourse.bass_utils import run_bass_kernel_spmd
    dims = _dims_full()
    nc = _get_nc("full")
    in_maps = host_prepare(inputs, dims)
    res = run_bass_kernel_spmd(nc, in_maps, core_ids=list(range(NCORES)))
    outs = [r["out"].astype(np.float32) for r in res.results]
    return np.concatenate(outs, axis=2)


# revision 33
# speedup vs baseline: 1.0765x; 1.0765x over previous
"""Self-contained Trainium2 Bass kernel for the sparse point-attention module.

Strategy: shard the point dimension n across the 8 NeuronCores (512 points
each, both batch entries on every core).  Each core gets the full `pos`
(tiny) so the KNN is purely local; everything else is data-parallel and no
collectives are needed.

Per-core pipeline (per batch b):
  Phase A (all 4096 points):  qfull = Wq@pos+bq,  pefull = MLP_p(qfull)
  Phase B (per 128-query tile):
     - dneg[i,j] = 2 p_i.p_j - |p_j|^2 via one fp32r matmul (K=4 with a
       ones row); top-16 neighbours via DVE max8 / max_index / match_replace
       (tie behaviour matches jax.lax.top_k exactly).
     - neighbour q/pe columns fetched from qfull/pefull with gpsimd
       ap_gather (the 16-NN index list is rewrapped with a PE transpose +
       tiled-identity broadcast matmul).
     - k_f/v convs, a-branch MLP (bf16 matmuls), softmax over the 16
       neighbours expressed as (sum e*w)/(sum e), final 1x1 conv.

Algebraic folds done on the host: BN (eval mode) into Wa1/Wp1+biases;
qk_rel*pe+pe = (q - k_f + 1)*pe with (1 - bk) folded into the k_f eviction;
ba2 dropped (softmax-invariant); all weights pre-transposed for the PE.
"""

import numpy as np
import ml_dtypes

BF16 = ml_dtypes.bfloat16

# ---- problem dimensions (hardcoded, must match the grader's inputs) ----
B = 2
CIN = 128
N = 4096
KK = 16          # neighbours
DIM = 256
PHID = 64
AHID = 1024
NCORES = 8
NLOC = N // NCORES
BN_EPS = 1e-5
NEG_BIG = -1e30


def _dims_full():
    return dict(B=B, CIN=CIN, N=N, KK=KK, DIM=DIM, PHID=PHID, AHID=AHID,
                NLOC=NLOC)


def build_nc(dims):
    """Build the (single, SPMD) Bass program for one core's shard."""
    import concourse.bass as bass
    import concourse.mybir as mybir
    import concourse.tile as tile
    from concourse import bacc
    from concourse.bass import ts

    fp32 = mybir.dt.float32
    fp32r = mybir.dt.float32r
    bf16 = mybir.dt.bfloat16
    u16 = mybir.dt.uint16
    i16 = mybir.dt.int16
    AF = mybir.ActivationFunctionType
    OP = mybir.AluOpType
    AX = mybir.AxisListType

    Bn = dims["B"]; CINn = dims["CIN"]; Nn = dims["N"]; KKn = dims["KK"]
    DIMn = dims["DIM"]; PHIDn = dims["PHID"]; AHIDn = dims["AHID"]
    NLOCn = dims["NLOC"]

    QT = min(128, NLOCn)              # queries per KNN tile
    NQT = NLOCn // QT                 # KNN tiles per batch
    CHUNK = 512                       # matmul column chunk (n,k cols)
    CQ = CHUNK // KKn                 # queries per chunk (32)
    NCH_TILE = (QT * KKn) // CHUNK    # chunks per KNN tile
    NCH_D = Nn // 512                 # 512-col chunks of the distance row
    DM = DIMn // 128                  # feature tiles (2)
    AM = AHIDn // 128                 # a-hidden tiles (8)
    KA1 = DIMn // 128                 # contraction tiles for a1 (2)

    nc = bacc.Bacc()

    # ---- DRAM parameters ----
    key_r = nc.declare_dram_parameter("key_r", [Bn, CINn, NLOCn * KKn], bf16, isOutput=False)
    val_r = nc.declare_dram_parameter("val_r", [Bn, CINn, NLOCn * KKn], bf16, isOutput=False)
    paug_lhs = nc.declare_dram_parameter("paug_lhs", [Bn, 11, NLOCn], bf16, isOutput=False)
    paug_rhs = nc.declare_dram_parameter("paug_rhs", [Bn, 11, Nn], bf16, isOutput=False)
    pq_rhs = nc.declare_dram_parameter("pq_rhs", [Bn, 4, Nn], bf16, isOutput=False)
    WkTn_d = nc.declare_dram_parameter("WkTn", [CINn, DIMn], bf16, isOutput=False)
    WvT_d = nc.declare_dram_parameter("WvT", [CINn, DIMn], bf16, isOutput=False)
    WqTb_d = nc.declare_dram_parameter("WqTb", [4, DIMn], bf16, isOutput=False)
    Wp1T_d = nc.declare_dram_parameter("Wp1T", [128, DM, PHIDn], bf16, isOutput=False)
    Wp2T_d = nc.declare_dram_parameter("Wp2T", [PHIDn, DIMn], bf16, isOutput=False)
    Wa1T_d = nc.declare_dram_parameter("Wa1T", [128, KA1, AHIDn], bf16, isOutput=False)
    Wa2T_d = nc.declare_dram_parameter("Wa2T", [128, AM, DIMn], bf16, isOutput=False)
    WeT_d = nc.declare_dram_parameter("WeT", [128, DM, DIMn], bf16, isOutput=False)
    negbk1_d = nc.declare_dram_parameter("negbk1", [128, DM], fp32, isOutput=False)
    bv_d = nc.declare_dram_parameter("bvf", [128, DM], fp32, isOutput=False)
    bp1_d = nc.declare_dram_parameter("bp1f", [PHIDn, 1], fp32, isOutput=False)
    bp2_d = nc.declare_dram_parameter("bp2f", [128, DM], fp32, isOutput=False)
    ba1_d = nc.declare_dram_parameter("ba1f", [128, AM], fp32, isOutput=False)
    be_d = nc.declare_dram_parameter("bef", [128, DM], fp32, isOutput=False)
    teye_d = nc.declare_dram_parameter("teye16", [16, 128], fp32, isOutput=False)
    eye_d = nc.declare_dram_parameter("eye128", [128, 128], fp32, isOutput=False)
    out_d = nc.declare_dram_parameter("out", [Bn, DIMn, NLOCn], fp32, isOutput=True)

    with tile.TileContext(nc) as tc:
        with (
            tc.tile_pool(name="wpool", bufs=1) as wpool,
            tc.tile_pool(name="bpool", bufs=1) as bpool,
            tc.tile_pool(name="dpool", bufs=1) as dpool,
            tc.tile_pool(name="kpool", bufs=2) as kpool,
            tc.tile_pool(name="cpool", bufs=2) as cpool,
            tc.tile_pool(name="c1pool", bufs=1) as c1pool,
            tc.tile_pool(name="papool", bufs=2) as papool,
            tc.tile_pool(name="psmisc", bufs=2, space="PSUM") as psmisc,
            tc.tile_pool(name="pskfv", bufs=1, space="PSUM") as pskfv,
            tc.tile_pool(name="psa1", bufs=2, space="PSUM") as psa1,
            tc.tile_pool(name="psa2", bufs=1, space="PSUM") as psa2,
        ):
            # ---- load weights / constants once ----
            WkTn = wpool.tile([CINn, DIMn], bf16)
            WvT = wpool.tile([CINn, DIMn], bf16)
            WqTb = wpool.tile([4, DIMn], bf16)
            Wp1T = wpool.tile([128, DM, PHIDn], bf16)
            Wp2T = wpool.tile([PHIDn, DIMn], bf16)
            Wa1T = wpool.tile([128, KA1, AHIDn], bf16)
            Wa2T = wpool.tile([128, AM, DIMn], bf16)
            WeT = wpool.tile([128, DM, DIMn], bf16)
            negbk1 = wpool.tile([128, DM], fp32)
            bvf = wpool.tile([128, DM], fp32)
            bp1f = wpool.tile([PHIDn, 1], fp32)
            bp2f = wpool.tile([128, DM], fp32)
            ba1f = wpool.tile([128, AM], fp32)
            bef = wpool.tile([128, DM], fp32)
            teye16 = wpool.tile([16, 128], fp32)
            eye128 = wpool.tile([128, 128], fp32)
            for sb, dr in [(WkTn, WkTn_d), (WvT, WvT_d), (WqTb, WqTb_d),
                           (Wp1T, Wp1T_d), (Wp2T, Wp2T_d), (Wa1T, Wa1T_d),
                           (Wa2T, Wa2T_d), (WeT, WeT_d), (negbk1, negbk1_d),
                           (bvf, bv_d), (bp1f, bp1_d), (bp2f, bp2_d),
                           (ba1f, ba1_d), (bef, be_d), (teye16, teye_d),
                           (eye128, eye_d)]:
                nc.sync.dma_start(out=sb[:], in_=dr[:])

            for b in range(Bn):
                # ============== Phase A: qfull / pefull on all N points =====
                pq_sb = papool.tile([4, Nn], bf16, tag="pq_sb")
                nc.sync.dma_start(out=pq_sb[:], in_=pq_rhs[b])
                prhs_sb = papool.tile([11, Nn], bf16, tag="prhs_sb")
                nc.sync.dma_start(out=prhs_sb[:], in_=paug_rhs[b])
                plhs_sb = papool.tile([11, NLOCn], bf16, tag="plhs_sb")
                nc.sync.dma_start(out=plhs_sb[:], in_=paug_lhs[b])

                qf32 = bpool.tile([128, DM, Nn], fp32, tag="qf32")
                pef32 = bpool.tile([128, DM, Nn], fp32, tag="pef32")
                aggsb = bpool.tile([128, DM, NLOCn], bf16, tag="aggsb")

                for nch in range(Nn // 512):
                    # q chunk: qf32 gets q + (1 - bk) (r-path); qbfc true q
                    qbfc = c1pool.tile([128, DM, 512], bf16, tag="qbfc")
                    for m in range(DM):
                        qps = psmisc.tile([128, 512], fp32, tag="ps")
                        nc.tensor.matmul(qps[:], WqTb[:, ts(m, 128)],
                                         pq_sb[:, ts(nch, 512)])
                        nc.scalar.activation(qf32[:, m, ts(nch, 512)], qps[:],
                                             AF.Identity,
                                             bias=negbk1[:, m:m + 1])
                        nc.vector.tensor_copy(qbfc[:, m, :], qps[:])
                    p1ps = psmisc.tile([PHIDn, 512], fp32, tag="ps")
                    for kt in range(DM):
                        nc.tensor.matmul(
                            p1ps[:], Wp1T[:, kt, :], qbfc[:, kt, :],
                            start=(kt == 0), stop=(kt == DM - 1))
                    pe1c = c1pool.tile([PHIDn, 512], bf16, tag="pe1c")
                    nc.scalar.activation(pe1c[:], p1ps[:],
                                         AF.Relu, bias=bp1f[:, 0:1])
                    for m in range(DM):
                        p2ps = psmisc.tile([128, 512], fp32, tag="ps")
                        nc.tensor.matmul(p2ps[:], Wp2T[:, ts(m, 128)], pe1c[:])
                        nc.vector.tensor_scalar_add(pef32[:, m, ts(nch, 512)],
                                                    p2ps[:], bp2f[:, m:m + 1])

                # ============== Phase B: per query tile ======================
                for t in range(NQT):
                    # ---- distance rows (negated, +sq_j folded in) ----
                    dsb = dpool.tile([QT, Nn], fp32, tag="dsb")
                    for nch in range(NCH_D):
                        dps = psmisc.tile([QT, 512], fp32, tag="ps")
                        nc.tensor.matmul(
                            dps[:], plhs_sb[:, ts(t, QT)],
                            prhs_sb[:, ts(nch, 512)])
                        nc.scalar.activation(dsb[:, ts(nch, 512)], dps[:],
                                             AF.Copy)

                    # ---- top-16 neighbours ----
                    v8a = kpool.tile([QT, 8], fp32, tag="v8a")
                    v8b = kpool.tile([QT, 8], fp32, tag="v8b")
                    idxg = kpool.tile([QT, 16], u16, tag="idxg")
                    idxf = kpool.tile([QT, 16], fp32, tag="idxf")
                    idxT = kpool.tile([16, QT], fp32, tag="idxT")
                    idxw = kpool.tile([128, QT], i16, tag="idxw")
                    if dims.get("debug") == "noknn":
                        nc.vector.memset(idxf[:], 1.0)
                    else:
                        nc.vector.max(out=v8a[:], in_=dsb[:])
                        nc.vector.max_index(out=idxg[:, 0:8], in_max=v8a[:],
                                            in_values=dsb[:])
                        nc.vector.match_replace(out=dsb[:], in_to_replace=v8a[:],
                                                in_values=dsb[:],
                                                imm_value=NEG_BIG)
                        nc.vector.max(out=v8b[:], in_=dsb[:])
                        nc.vector.max_index(out=idxg[:, 8:16], in_max=v8b[:],
                                            in_values=dsb[:])
                        nc.vector.tensor_copy(idxf[:], idxg[:])
                    trps = psmisc.tile([16, QT], fp32, tag="ps")
                    nc.tensor.transpose(trps[:], idxf[:], eye128[:QT, :QT])
                    nc.vector.tensor_copy(idxT[:], trps[:])
                    bcps = psmisc.tile([128, QT], fp32, tag="ps")
                    nc.tensor.matmul(bcps[:], teye16[:], idxT[:])
                    nc.vector.tensor_copy(idxw[:], bcps[:])

                    for c in range(NCH_TILE):
                        gc = t * NCH_TILE + c          # global chunk index
                        col0 = gc * CHUNK
                        # ---- gather neighbour q / pe columns ----
                        qg = cpool.tile([128, DM, CHUNK], fp32, tag="qg")
                        peg = cpool.tile([128, DM, CHUNK], fp32, tag="peg")
                        if dims.get("debug") == "nogather":
                            for m in range(DM):
                                nc.vector.tensor_copy(
                                    qg[:, m, :], qf32[:, m, 0:CHUNK])
                                nc.vector.tensor_copy(
                                    peg[:, m, :], pef32[:, m, 0:CHUNK])
                        else:
                            for m in range(DM):
                                nc.gpsimd.ap_gather(
                                    qg[:, m, :], qf32[:, m, :],
                                    idxw[:, c * CQ:(c + 1) * CQ],
                                    channels=128, num_elems=Nn, d=1,
                                    num_idxs=CHUNK)
                                nc.gpsimd.ap_gather(
                                    peg[:, m, :], pef32[:, m, :],
                                    idxw[:, c * CQ:(c + 1) * CQ],
                                    channels=128, num_elems=Nn, d=1,
                                    num_idxs=CHUNK)

                        # ---- k_f / v convs ----
                        kbf = cpool.tile([CINn, CHUNK], bf16, tag="kbf")
                        vbf = cpool.tile([CINn, CHUNK], bf16, tag="vbf")
                        nc.sync.dma_start(out=kbf[:],
                                          in_=key_r[b, :, col0:col0 + CHUNK])
                        nc.sync.dma_start(out=vbf[:],
                                          in_=val_r[b, :, col0:col0 + CHUNK])

                        rr = cpool.tile([128, DM, CHUNK], fp32, tag="rr")
                        vpe = cpool.tile([128, DM, CHUNK], fp32, tag="vpe")
                        a1in = cpool.tile([128, DM, CHUNK], bf16, tag="a1in")
                        kfps = pskfv.tile([128, DM, CHUNK], fp32, tag="kfv")
                        for m in range(DM):
                            nc.tensor.matmul(kfps[:, m, :],
                                             WkTn[:, ts(m, 128)], kbf[:])
                        # r = q_g(+1-bk) - k_f   (one fused op over both halves)
                        nc.vector.scalar_tensor_tensor(
                            rr[:], kfps[:], -1.0, qg[:],
                            op0=OP.mult, op1=OP.add)
                        vps = pskfv.tile([128, DM, CHUNK], fp32, tag="kfv")
                        for m in range(DM):
                            nc.tensor.matmul(vps[:, m, :],
                                             WvT[:, ts(m, 128)], vbf[:])
                        # vpe = v + pe (bv rides through softmax into bef)
                        nc.vector.scalar_tensor_tensor(
                            vpe[:], vps[:], 1.0, peg[:],
                            op0=OP.mult, op1=OP.add)
                        nc.vector.tensor_mul(a1in[:], rr[:], peg[:])

                        # ---- a-branch MLP ----
                        a1r = cpool.tile([128, AM, CHUNK], bf16, tag="a1r")
                        for mt in range(AM):
                            a1ps = psa1.tile([128, CHUNK], fp32, tag="a1")
                            for kt in range(KA1):
                                nc.tensor.matmul(
                                    a1ps[:], Wa1T[:, kt, ts(mt, 128)],
                                    a1in[:, kt, :],
                                    start=(kt == 0), stop=(kt == KA1 - 1))
                            nc.scalar.activation(a1r[:, mt, :], a1ps[:],
                                                 AF.Relu, bias=ba1f[:, mt:mt + 1])

                        ee = cpool.tile([128, DM, CHUNK], fp32, tag="ee")
                        esum = c1pool.tile([128, DM, CQ], fp32, tag="esum")
                        erec = c1pool.tile([128, DM, CQ], fp32, tag="erec")
                        aggc = c1pool.tile([128, DM, CQ], fp32, tag="aggc")
                        a2ps = psa2.tile([128, DM, CHUNK], fp32, tag="a2")
                        for m in range(DM):
                            for kt in range(AM):
                                nc.tensor.matmul(
                                    a2ps[:, m, :], Wa2T[:, kt, ts(m, 128)],
                                    a1r[:, kt, :],
                                    start=(kt == 0), stop=(kt == AM - 1))
                        # softmax over the 16 neighbours as sum(e*w)/sum(e)
                        nc.scalar.activation(ee[:], a2ps[:], AF.Exp)
                        nc.vector.tensor_reduce(
                            esum[:],
                            ee[:].rearrange("p m (g k) -> p m g k", k=KKn),
                            axis=AX.X, op=OP.add)
                        nc.vector.reciprocal(erec[:], esum[:])
                        nc.vector.tensor_tensor(ee[:], ee[:], vpe[:],
                                                op=OP.mult)
                        nc.vector.tensor_reduce(
                            aggc[:],
                            ee[:].rearrange("p m (g k) -> p m g k", k=KKn),
                            axis=AX.X, op=OP.add)
                        nc.vector.tensor_mul(
                            aggsb[:, :, t * QT + c * CQ:t * QT + (c + 1) * CQ],
                            aggc[:], erec[:])

                # ---- final 1x1 conv over aggregated features ----
                for nloc0 in range(0, NLOCn, 512):
                    w = min(512, NLOCn - nloc0)
                    for m in range(DM):
                        yps = psmisc.tile([128, 512], fp32, tag="ps")
                        for kt in range(DM):
                            nc.tensor.matmul(
                                yps[:, :w], WeT[:, kt, ts(m, 128)],
                                aggsb[:, kt, nloc0:nloc0 + w],
                                start=(kt == 0), stop=(kt == DM - 1))
                        yev = c1pool.tile([128, 512], fp32, tag="yev")
                        nc.vector.tensor_scalar_add(yev[:, :w], yps[:, :w],
                                                    bef[:, m:m + 1])
                        nc.sync.dma_start(
                            out=out_d[b, ts(m, 128), nloc0:nloc0 + w],
                            in_=yev[:, :w])

    nc.finalize()   # Bacc.finalize: wait legalization, library loads, ISA codegen
    return nc


def host_prepare(inputs, dims, ncores=NCORES):
    """Fold BN/biases into weights, pre-transpose for the PE, shard by n."""
    d = dims
    f32 = np.float32
    key = np.asarray(inputs["key"], f32)
    values = np.asarray(inputs["values"], f32)
    pos = np.asarray(inputs["pos"], f32)
    g = lambda n: np.asarray(inputs[n], f32)

    Wk, bk = g("Wk"), g("bk")
    Wq, bq = g("Wq"), g("bq")
    Wv, bv = g("Wv"), g("bv")
    Wp1, bp1 = g("Wp1"), g("bp1")
    Wp2, bp2 = g("Wp2"), g("bp2")
    Wa1, ba1 = g("Wa1"), g("ba1")
    Wa2 = g("Wa2")
    We, be = g("We"), g("be")

    p_sc = g("p_gamma") / np.sqrt(g("p_var") + f32(BN_EPS))
    Wp1f = (Wp1 * p_sc[:, None]).astype(f32)
    bp1f = (bp1 * p_sc + g("p_beta") - g("p_mean") * p_sc).astype(f32)
    a_sc = g("a_gamma") / np.sqrt(g("a_var") + f32(BN_EPS))
    Wa1f = (Wa1 * a_sc[:, None]).astype(f32)
    ba1f = (ba1 * a_sc + g("a_beta") - g("a_mean") * a_sc).astype(f32)

    DM = d["DIM"] // 128
    AM = d["AHID"] // 128
    KA1 = d["DIM"] // 128

    def colsplit(v, nt):  # (nt*128,) -> (128, nt)
        return np.ascontiguousarray(v.reshape(nt, 128).T).astype(f32)

    common = {
        "WkTn": np.ascontiguousarray(Wk.T).astype(BF16),
        "WvT": np.ascontiguousarray(Wv.T).astype(BF16),
        "WqTb": np.ascontiguousarray(
            np.concatenate([Wq.T, bq[None, :]], 0)).astype(BF16),
        "Wp1T": np.ascontiguousarray(
            Wp1f.T.reshape(KA1, 128, d["PHID"]).transpose(1, 0, 2)).astype(BF16),
        "Wp2T": np.ascontiguousarray(Wp2.T).astype(BF16),
        "Wa1T": np.ascontiguousarray(
            Wa1f.T.reshape(KA1, 128, d["AHID"]).transpose(1, 0, 2)).astype(BF16),
        "Wa2T": np.ascontiguousarray(
            Wa2.T.reshape(AM, 128, d["DIM"]).transpose(1, 0, 2)).astype(BF16),
        "WeT": np.ascontiguousarray(
            We.T.reshape(DM, 128, d["DIM"]).transpose(1, 0, 2)).astype(BF16),
        "negbk1": colsplit((1.0 - bk).astype(f32), DM),
        "bvf": colsplit(bv, DM),
        "bp1f": bp1f.reshape(d["PHID"], 1).astype(f32),
        "bp2f": colsplit(bp2, DM),
        "ba1f": colsplit(ba1f, AM),
        "bef": colsplit((We @ bv + be).astype(f32), DM),
        "teye16": np.ascontiguousarray(
            np.tile(np.eye(16, dtype=f32), (1, 8))),
        "eye128": np.eye(128, dtype=f32),
    }

    # hi/lo bf16 split of pos and |p|^2 for the exact-enough distance matmul:
    # dneg = 2(hi_i+lo_i).(hi_j+lo_j) - sq_j, dropping only the lo.lo term.
    sq = (pos * pos).sum(axis=1).astype(f32)              # (B, N)
    pos_hi = pos.astype(BF16)
    pos_lo = (pos - pos_hi.astype(f32)).astype(BF16)
    sq_hi = sq.astype(BF16)
    sq_lo = (sq - sq_hi.astype(f32)).astype(BF16)
    ones_n = np.ones((d["B"], 1, d["N"]), f32)
    paug_rhs = np.concatenate(
        [2.0 * pos_hi.astype(f32), 2.0 * pos_lo.astype(f32),
         2.0 * pos_hi.astype(f32), -sq_hi.astype(f32)[:, None, :],
         -sq_lo.astype(f32)[:, None, :]], 1).astype(BF16)
    pq_rhs = np.concatenate([pos, ones_n], 1).astype(BF16)

    in_maps = []
    for cid in range(ncores):
        n0 = cid * d["NLOC"]
        n1 = n0 + d["NLOC"]
        m = dict(common)
        m["key_r"] = np.ascontiguousarray(key[:, :, n0:n1, :]).reshape(
            d["B"], d["CIN"], d["NLOC"] * d["KK"]).astype(BF16)
        m["val_r"] = np.ascontiguousarray(values[:, :, n0:n1, :]).reshape(
            d["B"], d["CIN"], d["NLOC"] * d["KK"]).astype(BF16)
        m["paug_lhs"] = np.ascontiguousarray(np.concatenate(
            [pos_hi.astype(f32)[:, :, n0:n1], pos_hi.astype(f32)[:, :, n0:n1],
             pos_lo.astype(f32)[:, :, n0:n1],
             np.ones((d["B"], 2, d["NLOC"]), f32)], 1)).astype(BF16)
        m["paug_rhs"] = paug_rhs
        m["pq_rhs"] = pq_rhs
        in_maps.append(m)
    return in_maps


_NC_CACHE = {}


def _get_nc(dims_key):
    if dims_key not in _NC_CACHE:
        _NC_CACHE[dims_key] = build_nc(_dims_full())
    return _NC_CACHE[dims_key]


def kernel(**inputs):
    from concourse.bass_utils import run_bass_kernel_spmd
    dims = _dims_full()
    nc = _get_nc("full")
    in_maps = host_prepare(inputs, dims)
    res = run_bass_kernel_spmd(nc, in_maps, core_ids=list(range(NCORES)))
    outs = [r["out"].astype(np.float32) for r in res.results]
    return np.concatenate(outs, axis=2)


# revision 40
# speedup vs baseline: 2.0478x; 1.9022x over previous
"""Self-contained Trainium2 Bass kernel for the sparse point-attention module.

Strategy: shard the point dimension n across the 8 NeuronCores (512 points
each, both batch entries on every core).  Each core gets the full `pos`
(tiny) so the KNN is purely local; everything else is data-parallel and no
collectives are needed.

Per-core pipeline (per batch b):
  Phase A (all 4096 points):  qfull = Wq@pos+bq,  pefull = MLP_p(qfull)
  Phase B (per 128-query tile):
     - dneg[i,j] = 2 p_i.p_j - |p_j|^2 via one fp32r matmul (K=4 with a
       ones row); top-16 neighbours via DVE max8 / max_index / match_replace
       (tie behaviour matches jax.lax.top_k exactly).
     - neighbour q/pe columns fetched from qfull/pefull with gpsimd
       ap_gather (the 16-NN index list is rewrapped with a PE transpose +
       tiled-identity broadcast matmul).
     - k_f/v convs, a-branch MLP (bf16 matmuls), softmax over the 16
       neighbours expressed as (sum e*w)/(sum e), final 1x1 conv.

Algebraic folds done on the host: BN (eval mode) into Wa1/Wp1+biases;
qk_rel*pe+pe = (q - k_f + 1)*pe with (1 - bk) folded into the k_f eviction;
ba2 dropped (softmax-invariant); all weights pre-transposed for the PE.
"""

import numpy as np
import ml_dtypes

BF16 = ml_dtypes.bfloat16

# ---- problem dimensions (hardcoded, must match the grader's inputs) ----
B = 2
CIN = 128
N = 4096
KK = 16          # neighbours
DIM = 256
PHID = 64
AHID = 1024
NCORES = 8
NLOC = N // NCORES
BN_EPS = 1e-5
NEG_BIG = -1e30


def _dims_full():
    return dict(B=B, CIN=CIN, N=N, KK=KK, DIM=DIM, PHID=PHID, AHID=AHID,
                NLOC=NLOC)


def build_nc(dims):
    """Build the (single, SPMD) Bass program for one core's shard."""
    import concourse.bass as bass
    import concourse.mybir as mybir
    import concourse.tile as tile
    from concourse import bacc
    from concourse.bass import ts

    fp32 = mybir.dt.float32
    fp32r = mybir.dt.float32r
    bf16 = mybir.dt.bfloat16
    u16 = mybir.dt.uint16
    i16 = mybir.dt.int16
    AF = mybir.ActivationFunctionType
    OP = mybir.AluOpType
    AX = mybir.AxisListType

    Bn = dims["B"]; CINn = dims["CIN"]; Nn = dims["N"]; KKn = dims["KK"]
    DIMn = dims["DIM"]; PHIDn = dims["PHID"]; AHIDn = dims["AHID"]
    NLOCn = dims["NLOC"]

    QT = min(128, NLOCn)              # queries per KNN tile
    NQT = NLOCn // QT                 # KNN tiles per batch
    CHUNK = 512                       # matmul column chunk (n,k cols)
    CQ = CHUNK // KKn                 # queries per chunk (32)
    NCH_TILE = (QT * KKn) // CHUNK    # chunks per KNN tile
    NCH_D = Nn // 512                 # 512-col chunks of the distance row
    DM = DIMn // 128                  # feature tiles (2)
    AM = AHIDn // 128                 # a-hidden tiles (8)
    KA1 = DIMn // 128                 # contraction tiles for a1 (2)

    nc = bacc.Bacc()

    # ---- DRAM parameters ----
    key_r = nc.declare_dram_parameter("key_r", [Bn, CINn, NLOCn * KKn], bf16, isOutput=False)
    val_r = nc.declare_dram_parameter("val_r", [Bn, CINn, NLOCn * KKn], bf16, isOutput=False)
    paug_lhs = nc.declare_dram_parameter("paug_lhs", [Bn, 11, NLOCn], bf16, isOutput=False)
    paug_rhs = nc.declare_dram_parameter("paug_rhs", [Bn, 11, Nn], bf16, isOutput=False)
    pos16_d = nc.declare_dram_parameter("pos16", [Bn, 16, Nn], fp32, isOutput=False)
    WkTn_d = nc.declare_dram_parameter("WkTn", [CINn, DIMn], bf16, isOutput=False)
    WvT_d = nc.declare_dram_parameter("WvT", [CINn, DIMn], bf16, isOutput=False)
    WqTb_d = nc.declare_dram_parameter("WqTb", [4, DIMn], bf16, isOutput=False)
    Wp1T_d = nc.declare_dram_parameter("Wp1T", [128, DM, PHIDn], bf16, isOutput=False)
    Wp2T_d = nc.declare_dram_parameter("Wp2T", [PHIDn, DIMn], bf16, isOutput=False)
    Wa1T_d = nc.declare_dram_parameter("Wa1T", [128, KA1, AHIDn], bf16, isOutput=False)
    Wa2T_d = nc.declare_dram_parameter("Wa2T", [128, AM, DIMn], bf16, isOutput=False)
    WeT_d = nc.declare_dram_parameter("WeT", [128, DM, DIMn], bf16, isOutput=False)
    negbk1_d = nc.declare_dram_parameter("negbk1", [128, DM], fp32, isOutput=False)
    bp1_d = nc.declare_dram_parameter("bp1f", [PHIDn, 1], fp32, isOutput=False)
    bp2_d = nc.declare_dram_parameter("bp2f", [128, DM], fp32, isOutput=False)
    ba1_d = nc.declare_dram_parameter("ba1f", [128, AM], fp32, isOutput=False)
    be_d = nc.declare_dram_parameter("bef", [128, DM], fp32, isOutput=False)
    eye_d = nc.declare_dram_parameter("eye128", [128, 128], fp32, isOutput=False)
    out_d = nc.declare_dram_parameter("out", [Bn, DIMn, NLOCn], fp32, isOutput=True)

    with tile.TileContext(nc) as tc:
        with (
            tc.tile_pool(name="wpool", bufs=1) as wpool,
            tc.tile_pool(name="bpool", bufs=1) as bpool,
            tc.tile_pool(name="dpool", bufs=1) as dpool,
            tc.tile_pool(name="kpool", bufs=2) as kpool,
            tc.tile_pool(name="cpool", bufs=2) as cpool,
            tc.tile_pool(name="c1pool", bufs=1) as c1pool,
            tc.tile_pool(name="papool", bufs=2) as papool,
            tc.tile_pool(name="psmisc", bufs=2, space="PSUM") as psmisc,
            tc.tile_pool(name="pskfv", bufs=1, space="PSUM") as pskfv,
            tc.tile_pool(name="psa1", bufs=2, space="PSUM") as psa1,
            tc.tile_pool(name="psa2", bufs=1, space="PSUM") as psa2,
        ):
            # ---- load weights / constants once ----
            WkTn = wpool.tile([CINn, DIMn], bf16)
            WvT = wpool.tile([CINn, DIMn], bf16)
            WqTb = wpool.tile([4, DIMn], bf16)
            Wp1T = wpool.tile([128, DM, PHIDn], bf16)
            Wp2T = wpool.tile([PHIDn, DIMn], bf16)
            Wa1T = wpool.tile([128, KA1, AHIDn], bf16)
            Wa2T = wpool.tile([128, AM, DIMn], bf16)
            WeT = wpool.tile([128, DM, DIMn], bf16)
            negbk1 = wpool.tile([128, DM], fp32)
            bp1f = wpool.tile([PHIDn, 1], fp32)
            bp2f = wpool.tile([128, DM], fp32)
            ba1f = wpool.tile([128, AM], fp32)
            bef = wpool.tile([128, DM], fp32)
            eye128 = wpool.tile([128, 128], fp32)
            for sb, dr in [(WkTn, WkTn_d), (WvT, WvT_d), (WqTb, WqTb_d),
                           (Wp1T, Wp1T_d), (Wp2T, Wp2T_d), (Wa1T, Wa1T_d),
                           (Wa2T, Wa2T_d), (WeT, WeT_d), (negbk1, negbk1_d),
                           (bp1f, bp1_d), (bp2f, bp2_d),
                           (ba1f, ba1_d), (bef, be_d),
                           (eye128, eye_d)]:
                nc.sync.dma_start(out=sb[:], in_=dr[:])

            for b in range(Bn):
                prhs_sb = papool.tile([11, Nn], bf16, tag="prhs_sb")
                nc.sync.dma_start(out=prhs_sb[:], in_=paug_rhs[b])
                plhs_sb = papool.tile([11, NLOCn], bf16, tag="plhs_sb")
                nc.sync.dma_start(out=plhs_sb[:], in_=paug_lhs[b])
                pos16 = papool.tile([16, Nn], fp32, tag="pos16")
                nc.sync.dma_start(out=pos16[:], in_=pos16_d[b])

                aggsb = bpool.tile([128, DM, NLOCn], bf16, tag="aggsb")

                # ============== per query tile ==============================
                for t in range(NQT):
                    # ---- distance rows (negated, +sq_j folded in) ----
                    dsb = dpool.tile([QT, Nn], fp32, tag="dsb")
                    for nch in range(NCH_D):
                        dps = psmisc.tile([QT, 512], fp32, tag="ps")
                        nc.tensor.matmul(
                            dps[:], plhs_sb[:, ts(t, QT)],
                            prhs_sb[:, ts(nch, 512)])
                        nc.scalar.activation(dsb[:, ts(nch, 512)], dps[:],
                                             AF.Copy)

                    # ---- top-16 neighbours ----
                    v8a = kpool.tile([QT, 8], fp32, tag="v8a")
                    v8b = kpool.tile([QT, 8], fp32, tag="v8b")
                    idxg = kpool.tile([QT, 16], u16, tag="idxg")
                    idxf = kpool.tile([QT, 16], fp32, tag="idxf")
                    idxT = kpool.tile([16, QT], fp32, tag="idxT")
                    idxw16 = kpool.tile([16, QT], i16, tag="idxw16")
                    if dims.get("debug") == "noknn":
                        nc.vector.memset(idxf[:], 1.0)
                    else:
                        nc.vector.max(out=v8a[:], in_=dsb[:])
                        nc.vector.max_index(out=idxg[:, 0:8], in_max=v8a[:],
                                            in_values=dsb[:])
                        nc.vector.match_replace(out=dsb[:], in_to_replace=v8a[:],
                                                in_values=dsb[:],
                                                imm_value=NEG_BIG)
                        nc.vector.max(out=v8b[:], in_=dsb[:])
                        nc.vector.max_index(out=idxg[:, 8:16], in_max=v8b[:],
                                            in_values=dsb[:])
                        nc.vector.tensor_copy(idxf[:], idxg[:])
                    trps = psmisc.tile([16, QT], fp32, tag="ps")
                    nc.tensor.transpose(trps[:], idxf[:], eye128[:QT, :QT])
                    nc.vector.tensor_copy(idxT[:], trps[:])
                    nc.vector.tensor_copy(idxw16[:], idxT[:])

                    for c in range(NCH_TILE):
                        gc = t * NCH_TILE + c          # global chunk index
                        col0 = gc * CHUNK
                        # ---- gather neighbour positions (tiny), then
                        #      recompute q and pe for this chunk ----
                        posg = cpool.tile([16, CHUNK], fp32, tag="posg")
                        nc.gpsimd.ap_gather(
                            posg[:], pos16[:],
                            idxw16[:, c * CQ:(c + 1) * CQ],
                            channels=16, num_elems=Nn, d=1, num_idxs=CHUNK)
                        posgb = cpool.tile([16, CHUNK], bf16, tag="posgb")
                        nc.vector.tensor_copy(posgb[:], posg[:])

                        qg = cpool.tile([128, DM, CHUNK], fp32, tag="qg")
                        peg = cpool.tile([128, DM, CHUNK], fp32, tag="peg")
                        qbfc = c1pool.tile([128, DM, CHUNK], bf16, tag="qbfc")
                        for m in range(DM):
                            qps = psmisc.tile([128, CHUNK], fp32, tag="ps")
                            nc.tensor.matmul(qps[:], WqTb[:, ts(m, 128)],
                                             posgb[0:4, :])
                            # qg = q + (1-bk) for the r-path; qbfc = true q
                            nc.scalar.activation(qg[:, m, :], qps[:],
                                                 AF.Identity,
                                                 bias=negbk1[:, m:m + 1])
                            nc.vector.tensor_copy(qbfc[:, m, :], qps[:])
                        p1ps = psmisc.tile([PHIDn, CHUNK], fp32, tag="ps")
                        for kt in range(DM):
                            nc.tensor.matmul(
                                p1ps[:], Wp1T[:, kt, :], qbfc[:, kt, :],
                                start=(kt == 0), stop=(kt == DM - 1))
                        pe1c = c1pool.tile([PHIDn, CHUNK], bf16, tag="pe1c")
                        nc.scalar.activation(pe1c[:], p1ps[:],
                                             AF.Relu, bias=bp1f[:, 0:1])
                        for m in range(DM):
                            p2ps = psmisc.tile([128, CHUNK], fp32, tag="ps")
                            nc.tensor.matmul(p2ps[:], Wp2T[:, ts(m, 128)],
                                             pe1c[:])
                            nc.vector.tensor_scalar_add(peg[:, m, :], p2ps[:],
                                                        bp2f[:, m:m + 1])

                        # ---- k_f / v convs ----
                        kbf = cpool.tile([CINn, CHUNK], bf16, tag="kbf")
                        vbf = cpool.tile([CINn, CHUNK], bf16, tag="vbf")
                        nc.sync.dma_start(out=kbf[:],
                                          in_=key_r[b, :, col0:col0 + CHUNK])
                        nc.sync.dma_start(out=vbf[:],
                                          in_=val_r[b, :, col0:col0 + CHUNK])

                        rr = cpool.tile([128, DM, CHUNK], fp32, tag="rr")
                        vpe = cpool.tile([128, DM, CHUNK], fp32, tag="vpe")
                        a1in = cpool.tile([128, DM, CHUNK], bf16, tag="a1in")
                        kfps = pskfv.tile([128, DM, CHUNK], fp32, tag="kfv")
                        for m in range(DM):
                            nc.tensor.matmul(kfps[:, m, :],
                                             WkTn[:, ts(m, 128)], kbf[:])
                        # r = q_g(+1-bk) - k_f   (one fused op over both halves)
                        nc.vector.scalar_tensor_tensor(
                            rr[:], kfps[:], -1.0, qg[:],
                            op0=OP.mult, op1=OP.add)
                        vps = pskfv.tile([128, DM, CHUNK], fp32, tag="kfv")
                        for m in range(DM):
                            nc.tensor.matmul(vps[:, m, :],
                                             WvT[:, ts(m, 128)], vbf[:])
                        # vpe = v + pe (bv rides through softmax into bef)
                        nc.vector.scalar_tensor_tensor(
                            vpe[:], vps[:], 1.0, peg[:],
                            op0=OP.mult, op1=OP.add)
                        nc.vector.tensor_mul(a1in[:], rr[:], peg[:])

                        # ---- a-branch MLP ----
                        a1r = cpool.tile([128, AM, CHUNK], bf16, tag="a1r")
                        for mt in range(AM):
                            a1ps = psa1.tile([128, CHUNK], fp32, tag="a1")
                            for kt in range(KA1):
                                nc.tensor.matmul(
                                    a1ps[:], Wa1T[:, kt, ts(mt, 128)],
                                    a1in[:, kt, :],
                                    start=(kt == 0), stop=(kt == KA1 - 1))
                            nc.scalar.activation(a1r[:, mt, :], a1ps[:],
                                                 AF.Relu, bias=ba1f[:, mt:mt + 1])

                        ee = cpool.tile([128, DM, CHUNK], fp32, tag="ee")
                        esum = c1pool.tile([128, DM, CQ], fp32, tag="esum")
                        erec = c1pool.tile([128, DM, CQ], fp32, tag="erec")
                        aggc = c1pool.tile([128, DM, CQ], fp32, tag="aggc")
                        a2ps = psa2.tile([128, DM, CHUNK], fp32, tag="a2")
                        for m in range(DM):
                            for kt in range(AM):
                                nc.tensor.matmul(
                                    a2ps[:, m, :], Wa2T[:, kt, ts(m, 128)],
                                    a1r[:, kt, :],
                                    start=(kt == 0), stop=(kt == AM - 1))
                        # softmax over the 16 neighbours as sum(e*w)/sum(e)
                        nc.scalar.activation(ee[:], a2ps[:], AF.Exp)
                        nc.vector.tensor_reduce(
                            esum[:],
                            ee[:].rearrange("p m (g k) -> p m g k", k=KKn),
                            axis=AX.X, op=OP.add)
                        nc.vector.reciprocal(erec[:], esum[:])
                        nc.vector.tensor_tensor(ee[:], ee[:], vpe[:],
                                                op=OP.mult)
                        nc.vector.tensor_reduce(
                            aggc[:],
                            ee[:].rearrange("p m (g k) -> p m g k", k=KKn),
                            axis=AX.X, op=OP.add)
                        nc.vector.tensor_mul(
                            aggsb[:, :, t * QT + c * CQ:t * QT + (c + 1) * CQ],
                            aggc[:], erec[:])

                # ---- final 1x1 conv over aggregated features ----
                for nloc0 in range(0, NLOCn, 512):
                    w = min(512, NLOCn - nloc0)
                    for m in range(DM):
                        yps = psmisc.tile([128, 512], fp32, tag="ps")
                        for kt in range(DM):
                            nc.tensor.matmul(
                                yps[:, :w], WeT[:, kt, ts(m, 128)],
                                aggsb[:, kt, nloc0:nloc0 + w],
                                start=(kt == 0), stop=(kt == DM - 1))
                        yev = c1pool.tile([128, 512], fp32, tag="yev")
                        nc.vector.tensor_scalar_add(yev[:, :w], yps[:, :w],
                                                    bef[:, m:m + 1])
                        nc.sync.dma_start(
                            out=out_d[b, ts(m, 128), nloc0:nloc0 + w],
                            in_=yev[:, :w])

    nc.finalize()   # Bacc.finalize: wait legalization, library loads, ISA codegen
    return nc


def host_prepare(inputs, dims, ncores=NCORES):
    """Fold BN/biases into weights, pre-transpose for the PE, shard by n."""
    d = dims
    f32 = np.float32
    key = np.asarray(inputs["key"], f32)
    values = np.asarray(inputs["values"], f32)
    pos = np.asarray(inputs["pos"], f32)
    g = lambda n: np.asarray(inputs[n], f32)

    Wk, bk = g("Wk"), g("bk")
    Wq, bq = g("Wq"), g("bq")
    Wv, bv = g("Wv"), g("bv")
    Wp1, bp1 = g("Wp1"), g("bp1")
    Wp2, bp2 = g("Wp2"), g("bp2")
    Wa1, ba1 = g("Wa1"), g("ba1")
    Wa2 = g("Wa2")
    We, be = g("We"), g("be")

    p_sc = g("p_gamma") / np.sqrt(g("p_var") + f32(BN_EPS))
    Wp1f = (Wp1 * p_sc[:, None]).astype(f32)
    bp1f = (bp1 * p_sc + g("p_beta") - g("p_mean") * p_sc).astype(f32)
    a_sc = g("a_gamma") / np.sqrt(g("a_var") + f32(BN_EPS))
    Wa1f = (Wa1 * a_sc[:, None]).astype(f32)
    ba1f = (ba1 * a_sc + g("a_beta") - g("a_mean") * a_sc).astype(f32)

    DM = d["DIM"] // 128
    AM = d["AHID"] // 128
    KA1 = d["DIM"] // 128

    def colsplit(v, nt):  # (nt*128,) -> (128, nt)
        return np.ascontiguousarray(v.reshape(nt, 128).T).astype(f32)

    common = {
        "WkTn": np.ascontiguousarray(Wk.T).astype(BF16),
        "WvT": np.ascontiguousarray(Wv.T).astype(BF16),
        "WqTb": np.ascontiguousarray(
            np.concatenate([Wq.T, bq[None, :]], 0)).astype(BF16),
        "Wp1T": np.ascontiguousarray(
            Wp1f.T.reshape(KA1, 128, d["PHID"]).transpose(1, 0, 2)).astype(BF16),
        "Wp2T": np.ascontiguousarray(Wp2.T).astype(BF16),
        "Wa1T": np.ascontiguousarray(
            Wa1f.T.reshape(KA1, 128, d["AHID"]).transpose(1, 0, 2)).astype(BF16),
        "Wa2T": np.ascontiguousarray(
            Wa2.T.reshape(AM, 128, d["DIM"]).transpose(1, 0, 2)).astype(BF16),
        "WeT": np.ascontiguousarray(
            We.T.reshape(DM, 128, d["DIM"]).transpose(1, 0, 2)).astype(BF16),
        "negbk1": colsplit((1.0 - bk).astype(f32), DM),
        "bp1f": bp1f.reshape(d["PHID"], 1).astype(f32),
        "bp2f": colsplit(bp2, DM),
        "ba1f": colsplit(ba1f, AM),
        "bef": colsplit((We @ bv + be).astype(f32), DM),
        "eye128": np.eye(128, dtype=f32),
    }

    # hi/lo bf16 split of pos and |p|^2 for the exact-enough distance matmul:
    # dneg = 2(hi_i+lo_i).(hi_j+lo_j) - sq_j, dropping only the lo.lo term.
    sq = (pos * pos).sum(axis=1).astype(f32)              # (B, N)
    pos_hi = pos.astype(BF16)
    pos_lo = (pos - pos_hi.astype(f32)).astype(BF16)
    sq_hi = sq.astype(BF16)
    sq_lo = (sq - sq_hi.astype(f32)).astype(BF16)
    ones_n = np.ones((d["B"], 1, d["N"]), f32)
    paug_rhs = np.concatenate(
        [2.0 * pos_hi.astype(f32), 2.0 * pos_lo.astype(f32),
         2.0 * pos_hi.astype(f32), -sq_hi.astype(f32)[:, None, :],
         -sq_lo.astype(f32)[:, None, :]], 1).astype(BF16)
    pos16 = np.zeros((d["B"], 16, d["N"]), f32)
    pos16[:, 0:3] = pos
    pos16[:, 3] = 1.0

    in_maps = []
    for cid in range(ncores):
        n0 = cid * d["NLOC"]
        n1 = n0 + d["NLOC"]
        m = dict(common)
        m["key_r"] = np.ascontiguousarray(key[:, :, n0:n1, :]).reshape(
            d["B"], d["CIN"], d["NLOC"] * d["KK"]).astype(BF16)
        m["val_r"] = np.ascontiguousarray(values[:, :, n0:n1, :]).reshape(
            d["B"], d["CIN"], d["NLOC"] * d["KK"]).astype(BF16)
        m["paug_lhs"] = np.ascontiguousarray(np.concatenate(
            [pos_hi.astype(f32)[:, :, n0:n1], pos_hi.astype(f32)[:, :, n0:n1],
             pos_lo.astype(f32)[:, :, n0:n1],
             np.ones((d["B"], 2, d["NLOC"]), f32)], 1)).astype(BF16)
        m["paug_rhs"] = paug_rhs
        m["pos16"] = pos16
        in_maps.append(m)
    return in_maps


_NC_CACHE = {}


def _get_nc(dims_key):
    if dims_key not in _NC_CACHE:
        _NC_CACHE[dims_key] = build_nc(_dims_full())
    return _NC_CACHE[dims_key]


def kernel(**inputs):
    from concourse.bass_utils import run_bass_kernel_spmd
    dims = _dims_full()
    nc = _get_nc("full")
    in_maps = host_prepare(inputs, dims)
    res = run_bass_kernel_spmd(nc, in_maps, core_ids=list(range(NCORES)))
    outs = [r["out"].astype(np.float32) for r in res.results]
    return np.concatenate(outs, axis=2)


# revision 45
# speedup vs baseline: 2.8241x; 1.3791x over previous
"""Self-contained Trainium2 Bass kernel for the sparse point-attention module.

Strategy: shard the point dimension n across the 8 NeuronCores (512 points
each, both batch entries on every core).  Each core gets the full `pos`
(tiny) so the KNN is purely local; everything else is data-parallel and no
collectives are needed.

Per-core pipeline (per batch b):
  Phase A (all 4096 points):  qfull = Wq@pos+bq,  pefull = MLP_p(qfull)
  Phase B (per 128-query tile):
     - dneg[i,j] = 2 p_i.p_j - |p_j|^2 via one fp32r matmul (K=4 with a
       ones row); top-16 neighbours via DVE max8 / max_index / match_replace
       (tie behaviour matches jax.lax.top_k exactly).
     - neighbour q/pe columns fetched from qfull/pefull with gpsimd
       ap_gather (the 16-NN index list is rewrapped with a PE transpose +
       tiled-identity broadcast matmul).
     - k_f/v convs, a-branch MLP (bf16 matmuls), softmax over the 16
       neighbours expressed as (sum e*w)/(sum e), final 1x1 conv.

Algebraic folds done on the host: BN (eval mode) into Wa1/Wp1+biases;
qk_rel*pe+pe = (q - k_f + 1)*pe with (1 - bk) folded into the k_f eviction;
ba2 dropped (softmax-invariant); all weights pre-transposed for the PE.
"""

import numpy as np
import ml_dtypes

BF16 = ml_dtypes.bfloat16

# ---- problem dimensions (hardcoded, must match the grader's inputs) ----
B = 2
CIN = 128
N = 4096
KK = 16          # neighbours
DIM = 256
PHID = 64
AHID = 1024
NCORES = 8
NLOC = N // NCORES
BN_EPS = 1e-5
NEG_BIG = -1e30


def _dims_full():
    return dict(B=B, CIN=CIN, N=N, KK=KK, DIM=DIM, PHID=PHID, AHID=AHID,
                NLOC=NLOC)


def build_nc(dims):
    """Build the (single, SPMD) Bass program for one core's shard."""
    import concourse.bass as bass
    import concourse.mybir as mybir
    import concourse.tile as tile
    from concourse import bacc
    from concourse.bass import ts

    fp32 = mybir.dt.float32
    fp32r = mybir.dt.float32r
    bf16 = mybir.dt.bfloat16
    u16 = mybir.dt.uint16
    i16 = mybir.dt.int16
    AF = mybir.ActivationFunctionType
    OP = mybir.AluOpType
    AX = mybir.AxisListType

    Bn = dims["B"]; CINn = dims["CIN"]; Nn = dims["N"]; KKn = dims["KK"]
    DIMn = dims["DIM"]; PHIDn = dims["PHID"]; AHIDn = dims["AHID"]
    NLOCn = dims["NLOC"]

    QT = min(128, NLOCn)              # queries per KNN tile
    NQT = NLOCn // QT                 # KNN tiles per batch
    CHUNK = 512                       # matmul column chunk (n,k cols)
    CQ = CHUNK // KKn                 # queries per chunk (32)
    NCH_TILE = (QT * KKn) // CHUNK    # chunks per KNN tile
    NCH_D = Nn // 512                 # 512-col chunks of the distance row
    DM = DIMn // 128                  # feature tiles (2)
    AM = AHIDn // 128                 # a-hidden tiles (8)
    KA1 = DIMn // 128                 # contraction tiles for a1 (2)

    nc = bacc.Bacc()

    # ---- DRAM parameters ----
    key_r = nc.declare_dram_parameter("key_r", [Bn, CINn, NLOCn * KKn], bf16, isOutput=False)
    val_r = nc.declare_dram_parameter("val_r", [Bn, CINn, NLOCn * KKn], bf16, isOutput=False)
    paug_lhs = nc.declare_dram_parameter("paug_lhs", [Bn, 11, NLOCn], bf16, isOutput=False)
    paug_rhs = nc.declare_dram_parameter("paug_rhs", [Bn, 11, Nn], bf16, isOutput=False)
    pos16_d = nc.declare_dram_parameter("pos16", [Bn, 16, Nn], fp32, isOutput=False)
    WkTn_d = nc.declare_dram_parameter("WkTn", [CINn, DIMn], bf16, isOutput=False)
    WvT_d = nc.declare_dram_parameter("WvT", [CINn, DIMn], bf16, isOutput=False)
    WqTb_d = nc.declare_dram_parameter("WqTb", [4, DIMn], bf16, isOutput=False)
    Wp1T_d = nc.declare_dram_parameter("Wp1T", [128, DM, PHIDn], bf16, isOutput=False)
    Wp2T_d = nc.declare_dram_parameter("Wp2T", [PHIDn, DIMn], bf16, isOutput=False)
    Wa1T_d = nc.declare_dram_parameter("Wa1T", [128, KA1, AHIDn], bf16, isOutput=False)
    Wa2T_d = nc.declare_dram_parameter("Wa2T", [128, AM, DIMn], bf16, isOutput=False)
    WeT_d = nc.declare_dram_parameter("WeT", [128, DM, DIMn], bf16, isOutput=False)
    negbk1_d = nc.declare_dram_parameter("negbk1", [128, DM], fp32, isOutput=False)
    bp1_d = nc.declare_dram_parameter("bp1f", [PHIDn, 1], fp32, isOutput=False)
    bp2_d = nc.declare_dram_parameter("bp2f", [128, DM], fp32, isOutput=False)
    ba1_d = nc.declare_dram_parameter("ba1f", [128, AM], fp32, isOutput=False)
    be_d = nc.declare_dram_parameter("bef", [128, DM], fp32, isOutput=False)
    eye_d = nc.declare_dram_parameter("eye128", [128, 128], fp32, isOutput=False)
    out_d = nc.declare_dram_parameter("out", [Bn, DIMn, NLOCn], fp32, isOutput=True)

    with tile.TileContext(nc) as tc:
        with (
            tc.tile_pool(name="wpool", bufs=1) as wpool,
            tc.tile_pool(name="bpool", bufs=2) as bpool,
            tc.tile_pool(name="dpool", bufs=1) as dpool,
            tc.tile_pool(name="kpool", bufs=2) as kpool,
            tc.tile_pool(name="cpool", bufs=2) as cpool,
            tc.tile_pool(name="c1pool", bufs=1) as c1pool,
            tc.tile_pool(name="papool", bufs=2) as papool,
            tc.tile_pool(name="psmisc", bufs=2, space="PSUM") as psmisc,
            tc.tile_pool(name="pskfv", bufs=1, space="PSUM") as pskfv,
            tc.tile_pool(name="psa1", bufs=2, space="PSUM") as psa1,
            tc.tile_pool(name="psa2", bufs=1, space="PSUM") as psa2,
        ):
            # ---- load weights / constants once ----
            WkTn = wpool.tile([CINn, DIMn], bf16)
            WvT = wpool.tile([CINn, DIMn], bf16)
            WqTb = wpool.tile([4, DIMn], bf16)
            Wp1T = wpool.tile([128, DM, PHIDn], bf16)
            Wp2T = wpool.tile([PHIDn, DIMn], bf16)
            Wa1T = wpool.tile([128, KA1, AHIDn], bf16)
            Wa2T = wpool.tile([128, AM, DIMn], bf16)
            WeT = wpool.tile([128, DM, DIMn], bf16)
            negbk1 = wpool.tile([128, DM], fp32)
            bp1f = wpool.tile([PHIDn, 1], fp32)
            bp2f = wpool.tile([128, DM], fp32)
            ba1f = wpool.tile([128, AM], fp32)
            bef = wpool.tile([128, DM], fp32)
            eye128 = wpool.tile([128, 128], fp32)
            for sb, dr in [(WkTn, WkTn_d), (WvT, WvT_d), (WqTb, WqTb_d),
                           (Wp1T, Wp1T_d), (Wp2T, Wp2T_d), (Wa1T, Wa1T_d),
                           (Wa2T, Wa2T_d), (WeT, WeT_d), (negbk1, negbk1_d),
                           (bp1f, bp1_d), (bp2f, bp2_d),
                           (ba1f, ba1_d), (bef, be_d),
                           (eye128, eye_d)]:
                nc.sync.dma_start(out=sb[:], in_=dr[:])

            prhs_sbs, plhs_sbs, pos16s = [], [], []
            for b in range(Bn):
                prhs_sb = papool.tile([11, Nn], bf16, tag="prhs_sb")
                nc.sync.dma_start(out=prhs_sb[:], in_=paug_rhs[b])
                plhs_sb = papool.tile([11, NLOCn], bf16, tag="plhs_sb")
                nc.sync.dma_start(out=plhs_sb[:], in_=paug_lhs[b])
                pos16 = papool.tile([16, Nn], fp32, tag="pos16")
                nc.sync.dma_start(out=pos16[:], in_=pos16_d[b])
                prhs_sbs.append(prhs_sb); plhs_sbs.append(plhs_sb)
                pos16s.append(pos16)

            def emit_knn(b, t):
                """distance rows + top-16 + rewrapped gather indices."""
                dsb = dpool.tile([QT, Nn], fp32, tag="dsb")
                for nch in range(NCH_D):
                    dps = psmisc.tile([QT, 512], fp32, tag="ps")
                    nc.tensor.matmul(
                        dps[:], plhs_sbs[b][:, ts(t, QT)],
                        prhs_sbs[b][:, ts(nch, 512)])
                    nc.scalar.activation(dsb[:, ts(nch, 512)], dps[:],
                                         AF.Copy)
                v8a = kpool.tile([QT, 8], fp32, tag="v8a")
                v8b = kpool.tile([QT, 8], fp32, tag="v8b")
                idxg = kpool.tile([QT, 16], u16, tag="idxg")
                idxf = kpool.tile([QT, 16], fp32, tag="idxf")
                idxT = kpool.tile([16, QT], fp32, tag="idxT")
                idxw16 = kpool.tile([16, QT], i16, tag="idxw16")
                nc.vector.max(out=v8a[:], in_=dsb[:])
                nc.vector.max_index(out=idxg[:, 0:8], in_max=v8a[:],
                                    in_values=dsb[:])
                nc.vector.match_replace(out=dsb[:], in_to_replace=v8a[:],
                                        in_values=dsb[:], imm_value=NEG_BIG)
                nc.vector.max(out=v8b[:], in_=dsb[:])
                nc.vector.max_index(out=idxg[:, 8:16], in_max=v8b[:],
                                    in_values=dsb[:])
                nc.vector.tensor_copy(idxf[:], idxg[:])
                trps = psmisc.tile([16, QT], fp32, tag="ps")
                nc.tensor.transpose(trps[:], idxf[:], eye128[:QT, :QT])
                nc.vector.tensor_copy(idxT[:], trps[:])
                nc.vector.tensor_copy(idxw16[:], idxT[:])
                return idxw16

            tiles = [(b, t) for b in range(Bn) for t in range(NQT)]
            idxw16 = emit_knn(*tiles[0])
            for ti, (b, t) in enumerate(tiles):
                cur_idxw16 = idxw16
                if t == 0:
                    aggsb = bpool.tile([128, DM, NLOCn], bf16, tag="aggsb")
                # pipeline: next tile's distances+KNN run on PE/DVE while this
                # tile's chunks occupy PE with the MLP matmuls
                if ti + 1 < len(tiles):
                    idxw16 = emit_knn(*tiles[ti + 1])
                pos16 = pos16s[b]
                if True:
                    for c in range(NCH_TILE):
                        gc = t * NCH_TILE + c          # global chunk index
                        col0 = gc * CHUNK
                        # ---- gather neighbour positions (tiny), then
                        #      recompute q and pe for this chunk ----
                        posg = cpool.tile([16, CHUNK], fp32, tag="posg")
                        nc.gpsimd.ap_gather(
                            posg[:], pos16[:],
                            cur_idxw16[:, c * CQ:(c + 1) * CQ],
                            channels=16, num_elems=Nn, d=1, num_idxs=CHUNK)
                        posgb = cpool.tile([16, CHUNK], bf16, tag="posgb")
                        nc.vector.tensor_copy(posgb[:], posg[:])

                        qg = cpool.tile([128, DM, CHUNK], fp32, tag="qg")
                        peg = cpool.tile([128, DM, CHUNK], fp32, tag="peg")
                        qbfc = c1pool.tile([128, DM, CHUNK], bf16, tag="qbfc")
                        for m in range(DM):
                            qps = psmisc.tile([128, CHUNK], fp32, tag="ps")
                            nc.tensor.matmul(qps[:], WqTb[:, ts(m, 128)],
                                             posgb[0:4, :])
                            # qg = q + (1-bk) for the r-path; qbfc = true q
                            nc.scalar.activation(qg[:, m, :], qps[:],
                                                 AF.Identity,
                                                 bias=negbk1[:, m:m + 1])
                            nc.vector.tensor_copy(qbfc[:, m, :], qps[:])
                        p1ps = psmisc.tile([PHIDn, CHUNK], fp32, tag="ps")
                        for kt in range(DM):
                            nc.tensor.matmul(
                                p1ps[:], Wp1T[:, kt, :], qbfc[:, kt, :],
                                start=(kt == 0), stop=(kt == DM - 1))
                        pe1c = c1pool.tile([PHIDn, CHUNK], bf16, tag="pe1c")
                        nc.scalar.activation(pe1c[:], p1ps[:],
                                             AF.Relu, bias=bp1f[:, 0:1])
                        for m in range(DM):
                            p2ps = psmisc.tile([128, CHUNK], fp32, tag="ps")
                            nc.tensor.matmul(p2ps[:], Wp2T[:, ts(m, 128)],
                                             pe1c[:])
                            nc.vector.tensor_scalar_add(peg[:, m, :], p2ps[:],
                                                        bp2f[:, m:m + 1])

                        # ---- k_f / v convs ----
                        kbf = cpool.tile([CINn, CHUNK], bf16, tag="kbf")
                        vbf = cpool.tile([CINn, CHUNK], bf16, tag="vbf")
                        nc.sync.dma_start(out=kbf[:],
                                          in_=key_r[b, :, col0:col0 + CHUNK])
                        nc.sync.dma_start(out=vbf[:],
                                          in_=val_r[b, :, col0:col0 + CHUNK])

                        rr = cpool.tile([128, DM, CHUNK], fp32, tag="rr")
                        vpe = cpool.tile([128, DM, CHUNK], fp32, tag="vpe")
                        a1in = cpool.tile([128, DM, CHUNK], bf16, tag="a1in")
                        kfps = pskfv.tile([128, DM, CHUNK], fp32, tag="kfv")
                        for m in range(DM):
                            nc.tensor.matmul(kfps[:, m, :],
                                             WkTn[:, ts(m, 128)], kbf[:])
                        # r = q_g(+1-bk) - k_f   (one fused op over both halves)
                        nc.vector.scalar_tensor_tensor(
                            rr[:], kfps[:], -1.0, qg[:],
                            op0=OP.mult, op1=OP.add)
                        vps = pskfv.tile([128, DM, CHUNK], fp32, tag="kfv")
                        for m in range(DM):
                            nc.tensor.matmul(vps[:, m, :],
                                             WvT[:, ts(m, 128)], vbf[:])
                        # vpe = v + pe (bv rides through softmax into bef)
                        nc.vector.scalar_tensor_tensor(
                            vpe[:], vps[:], 1.0, peg[:],
                            op0=OP.mult, op1=OP.add)
                        nc.vector.tensor_mul(a1in[:], rr[:], peg[:])

                        # ---- a-branch MLP ----
                        a1r = cpool.tile([128, AM, CHUNK], bf16, tag="a1r")
                        for mt in range(AM):
                            a1ps = psa1.tile([128, CHUNK], fp32, tag="a1")
                            for kt in range(KA1):
                                nc.tensor.matmul(
                                    a1ps[:], Wa1T[:, kt, ts(mt, 128)],
                                    a1in[:, kt, :],
                                    start=(kt == 0), stop=(kt == KA1 - 1))
                            nc.scalar.activation(a1r[:, mt, :], a1ps[:],
                                                 AF.Relu, bias=ba1f[:, mt:mt + 1])

                        ee = cpool.tile([128, DM, CHUNK], fp32, tag="ee")
                        esum = c1pool.tile([128, DM, CQ], fp32, tag="esum")
                        erec = c1pool.tile([128, DM, CQ], fp32, tag="erec")
                        aggc = c1pool.tile([128, DM, CQ], fp32, tag="aggc")
                        a2ps = psa2.tile([128, DM, CHUNK], fp32, tag="a2")
                        for m in range(DM):
                            for kt in range(AM):
                                nc.tensor.matmul(
                                    a2ps[:, m, :], Wa2T[:, kt, ts(m, 128)],
                                    a1r[:, kt, :],
                                    start=(kt == 0), stop=(kt == AM - 1))
                        # softmax over the 16 neighbours as sum(e*w)/sum(e)
                        nc.scalar.activation(ee[:], a2ps[:], AF.Exp)
                        nc.vector.tensor_reduce(
                            esum[:],
                            ee[:].rearrange("p m (g k) -> p m g k", k=KKn),
                            axis=AX.X, op=OP.add)
                        nc.vector.reciprocal(erec[:], esum[:])
                        nc.vector.tensor_tensor(ee[:], ee[:], vpe[:],
                                                op=OP.mult)
                        nc.vector.tensor_reduce(
                            aggc[:],
                            ee[:].rearrange("p m (g k) -> p m g k", k=KKn),
                            axis=AX.X, op=OP.add)
                        nc.vector.tensor_mul(
                            aggsb[:, :, t * QT + c * CQ:t * QT + (c + 1) * CQ],
                            aggc[:], erec[:])

                # ---- final 1x1 conv once this batch's tiles are done ----
                if t == NQT - 1:
                    for nloc0 in range(0, NLOCn, 512):
                        w = min(512, NLOCn - nloc0)
                        for m in range(DM):
                            yps = psmisc.tile([128, 512], fp32, tag="ps")
                            for kt in range(DM):
                                nc.tensor.matmul(
                                    yps[:, :w], WeT[:, kt, ts(m, 128)],
                                    aggsb[:, kt, nloc0:nloc0 + w],
                                    start=(kt == 0), stop=(kt == DM - 1))
                            yev = c1pool.tile([128, 512], fp32, tag="yev")
                            nc.vector.tensor_scalar_add(yev[:, :w], yps[:, :w],
                                                        bef[:, m:m + 1])
                            nc.sync.dma_start(
                                out=out_d[b, ts(m, 128), nloc0:nloc0 + w],
                                in_=yev[:, :w])

    nc.finalize()   # Bacc.finalize: wait legalization, library loads, ISA codegen
    return nc


def host_prepare(inputs, dims, ncores=NCORES):
    """Fold BN/biases into weights, pre-transpose for the PE, shard by n."""
    d = dims
    f32 = np.float32
    key = np.asarray(inputs["key"], f32)
    values = np.asarray(inputs["values"], f32)
    pos = np.asarray(inputs["pos"], f32)
    g = lambda n: np.asarray(inputs[n], f32)

    Wk, bk = g("Wk"), g("bk")
    Wq, bq = g("Wq"), g("bq")
    Wv, bv = g("Wv"), g("bv")
    Wp1, bp1 = g("Wp1"), g("bp1")
    Wp2, bp2 = g("Wp2"), g("bp2")
    Wa1, ba1 = g("Wa1"), g("ba1")
    Wa2 = g("Wa2")
    We, be = g("We"), g("be")

    p_sc = g("p_gamma") / np.sqrt(g("p_var") + f32(BN_EPS))
    Wp1f = (Wp1 * p_sc[:, None]).astype(f32)
    bp1f = (bp1 * p_sc + g("p_beta") - g("p_mean") * p_sc).astype(f32)
    a_sc = g("a_gamma") / np.sqrt(g("a_var") + f32(BN_EPS))
    Wa1f = (Wa1 * a_sc[:, None]).astype(f32)
    ba1f = (ba1 * a_sc + g("a_beta") - g("a_mean") * a_sc).astype(f32)

    DM = d["DIM"] // 128
    AM = d["AHID"] // 128
    KA1 = d["DIM"] // 128

    def colsplit(v, nt):  # (nt*128,) -> (128, nt)
        return np.ascontiguousarray(v.reshape(nt, 128).T).astype(f32)

    common = {
        "WkTn": np.ascontiguousarray(Wk.T).astype(BF16),
        "WvT": np.ascontiguousarray(Wv.T).astype(BF16),
        "WqTb": np.ascontiguousarray(
            np.concatenate([Wq.T, bq[None, :]], 0)).astype(BF16),
        "Wp1T": np.ascontiguousarray(
            Wp1f.T.reshape(KA1, 128, d["PHID"]).transpose(1, 0, 2)).astype(BF16),
        "Wp2T": np.ascontiguousarray(Wp2.T).astype(BF16),
        "Wa1T": np.ascontiguousarray(
            Wa1f.T.reshape(KA1, 128, d["AHID"]).transpose(1, 0, 2)).astype(BF16),
        "Wa2T": np.ascontiguousarray(
            Wa2.T.reshape(AM, 128, d["DIM"]).transpose(1, 0, 2)).astype(BF16),
        "WeT": np.ascontiguousarray(
            We.T.reshape(DM, 128, d["DIM"]).transpose(1, 0, 2)).astype(BF16),
        "negbk1": colsplit((1.0 - bk).astype(f32), DM),
        "bp1f": bp1f.reshape(d["PHID"], 1).astype(f32),
        "bp2f": colsplit(bp2, DM),
        "ba1f": colsplit(ba1f, AM),
        "bef": colsplit((We @ bv + be).astype(f32), DM),
        "eye128": np.eye(128, dtype=f32),
    }

    # hi/lo bf16 split of pos and |p|^2 for the exact-enough distance matmul:
    # dneg = 2(hi_i+lo_i).(hi_j+lo_j) - sq_j, dropping only the lo.lo term.
    sq = (pos * pos).sum(axis=1).astype(f32)              # (B, N)
    pos_hi = pos.astype(BF16)
    pos_lo = (pos - pos_hi.astype(f32)).astype(BF16)
    sq_hi = sq.astype(BF16)
    sq_lo = (sq - sq_hi.astype(f32)).astype(BF16)
    ones_n = np.ones((d["B"], 1, d["N"]), f32)
    paug_rhs = np.concatenate(
        [2.0 * pos_hi.astype(f32), 2.0 * pos_lo.astype(f32),
         2.0 * pos_hi.astype(f32), -sq_hi.astype(f32)[:, None, :],
         -sq_lo.astype(f32)[:, None, :]], 1).astype(BF16)
    pos16 = np.zeros((d["B"], 16, d["N"]), f32)
    pos16[:, 0:3] = pos
    pos16[:, 3] = 1.0

    in_maps = []
    for cid in range(ncores):
        n0 = cid * d["NLOC"]
        n1 = n0 + d["NLOC"]
        m = dict(common)
        m["key_r"] = np.ascontiguousarray(key[:, :, n0:n1, :]).reshape(
            d["B"], d["CIN"], d["NLOC"] * d["KK"]).astype(BF16)
        m["val_r"] = np.ascontiguousarray(values[:, :, n0:n1, :]).reshape(
            d["B"], d["CIN"], d["NLOC"] * d["KK"]).astype(BF16)
        m["paug_lhs"] = np.ascontiguousarray(np.concatenate(
            [pos_hi.astype(f32)[:, :, n0:n1], pos_hi.astype(f32)[:, :, n0:n1],
             pos_lo.astype(f32)[:, :, n0:n1],
             np.ones((d["B"], 2, d["NLOC"]), f32)], 1)).astype(BF16)
        m["paug_rhs"] = paug_rhs
        m["pos16"] = pos16
        in_maps.append(m)
    return in_maps


_NC_CACHE = {}


def _get_nc(dims_key):
    if dims_key not in _NC_CACHE:
        _NC_CACHE[dims_key] = build_nc(_dims_full())
    return _NC_CACHE[dims_key]


def kernel(**inputs):
    from concourse.bass_utils import run_bass_kernel_spmd
    dims = _dims_full()
    nc = _get_nc("full")
    in_maps = host_prepare(inputs, dims)
    res = run_bass_kernel_spmd(nc, in_maps, core_ids=list(range(NCORES)))
    outs = [r["out"].astype(np.float32) for r in res.results]
    return np.concatenate(outs, axis=2)
